# revision 5
# baseline (speedup 1.0000x reference)
# Causal self-attention kernel for 8 Trainium2 NeuronCores (Bass/Tile).
#
# Sharding: core c -> batch b = c//4, head group g = c%4 (heads 4g..4g+3).
# Each core computes qkv projection for its batch restricted to its heads
# (column-sharded Wqkv), rope, causal flash attention for its 4 heads, and a
# row-sharded output projection producing a partial [S, D] f32 output.  Host
# sums the 4 partials per batch and adds bout.
#
# Device-side layout notes:
#  * All matmul inputs are bf16 (fp32 matmul is 4x slower on the PE); all
#    accumulation is f32 in PSUM.
#  * x is pre-transposed on host to xT [D, S] so the contraction dim (D) lands
#    on SBUF partitions without any on-device transpose.
#  * q/k are produced directly transposed: qT/kT [head dims, S].  Within each
#    head the 64 dims are permuted to [evens(32), odds(32)] so rope becomes
#    rot = x*P + swap32(x)*Q with per-row tables P/Q (host-built) and swap32
#    done by 2 SBUF->SBUF DMAs.
#  * Scores are computed transposed, sT[k, q] = k . q, via two accumulating
#    K=32 matmuls (even dims + odd dims) in distinct PE row groups.
#  * Softmax without max-subtraction (scores ~ N(0,1), exp is safe in f32):
#    p = exp(s/8) straight out of PSUM on the scalar engine (bf16 out).
#  * v_ext [k, 65] carries a ones-column so the PV matmul accumulates the
#    softmax denominator as row 64 of oT [65, q].  Normalization:
#    reciprocal (DVE) -> partition_broadcast (GpSimd) -> multiply (DVE).
#  * Output projection: y[q, n] = sum_d aT[d, q] * Wout[d, n], q-tiled.

import numpy as np
import ml_dtypes

import concourse.bass as bass
import concourse.tile as tile
import concourse.mybir as mybir
from concourse import bacc
from concourse.bass import ts, ds
from concourse.bass_utils import run_bass_kernel_spmd

BF16 = mybir.dt.bfloat16
F32 = mybir.dt.float32
AF = mybir.ActivationFunctionType
ALU = mybir.AluOpType

B, S, D = 2, 2048, 1024
H, HD = 16, 64
NCORES = 8
HPC = 4            # heads per core
NT = S // 128      # 16 seq tiles
SCALE = HD ** -0.5
MASK_NEG = -30000.0

# Module-level knobs / results (used by test.py).
TRACE = False
LAST_RESULTS = None


def _body(ctx, tc, ins, outs):
    nc = tc.nc
    xT, wqk, wv, wout, bqk, bvb, ropeP, ropeQ = ins
    (y,) = outs

    # ---- persistent SBUF pools ----
    p_x = ctx.enter_context(tc.tile_pool(name="x", bufs=1))
    p_w = ctx.enter_context(tc.tile_pool(name="w", bufs=1))
    p_cst = ctx.enter_context(tc.tile_pool(name="cst", bufs=1))
    p_qk = ctx.enter_context(tc.tile_pool(name="qk", bufs=1))
    p_vx = ctx.enter_context(tc.tile_pool(name="vx", bufs=1))
    p_aT = ctx.enter_context(tc.tile_pool(name="aT", bufs=1))
    p_tmp = ctx.enter_context(tc.tile_pool(name="tmp", bufs=2))
    p_p = ctx.enter_context(tc.tile_pool(name="p", bufs=4))
    p_r = ctx.enter_context(tc.tile_pool(name="r", bufs=2))
    p_y = ctx.enter_context(tc.tile_pool(name="y", bufs=4))

    # ---- load inputs ----
    x_sb = []
    for kc in range(8):
        t = p_x.tile([128, S], BF16, tag=f"x{kc}")
        nc.sync.dma_start(t[:, :], xT[ts(kc, 128), :])
        x_sb.append(t)
    wqk_sb = []
    for kc in range(8):
        t = p_w.tile([128, 512], BF16, tag=f"wqk{kc}")
        nc.sync.dma_start(t[:, :], wqk[ts(kc, 128), :])
        wqk_sb.append(t)
    wv_sb = []
    for kc in range(8):
        t = p_w.tile([128, 256], BF16, tag=f"wv{kc}")
        nc.sync.dma_start(t[:, :], wv[ts(kc, 128), :])
        wv_sb.append(t)
    wout_sb = []
    for kc in range(2):
        t = p_w.tile([128, D], BF16, tag=f"wout{kc}")
        nc.sync.dma_start(t[:, :], wout[ts(kc, 128), :])
        wout_sb.append(t)
    tabP = p_cst.tile([128, S], F32, tag="tabP")
    nc.sync.dma_start(tabP[:, :], ropeP[:, :])
    tabQ = p_cst.tile([128, S], F32, tag="tabQ")
    nc.sync.dma_start(tabQ[:, :], ropeQ[:, :])
    bqk_sb = []
    for mc in range(4):
        t = p_cst.tile([128, 1], F32, tag=f"bqk{mc}")
        nc.sync.dma_start(t[:, :], bqk[ts(mc, 128)].rearrange("(p o) -> p o", o=1))
        bqk_sb.append(t)
    bvb_sb = p_cst.tile([128, 256], F32, tag="bvb")
    nc.sync.dma_start(bvb_sb[:, :], bvb[:, :])

    # causal mask-add tile for diagonal blocks, layout [k_local, q_local]:
    # 0 where k<=q, MASK_NEG where k>q (i.e. col - row < 0).
    iota_t = p_cst.tile([128, 128], mybir.dt.int32, tag="iota")
    nc.gpsimd.iota(iota_t[:, :], pattern=[[1, 128]], base=0, channel_multiplier=-1)
    m01 = p_cst.tile([128, 128], F32, tag="m01")
    nc.vector.tensor_scalar(m01[:, :], iota_t[:, :], 0, None, op0=ALU.is_lt)
    maskneg = p_cst.tile([128, 128], F32, tag="maskneg")
    nc.vector.tensor_scalar_mul(maskneg[:, :], m01[:, :], MASK_NEG)

    qk_sb = []   # [q01, q23, k01, k23], bf16 [128, S] each (post-rope)
    for mc in range(4):
        qk_sb.append(p_qk.tile([128, S], BF16, tag=f"qkT{mc}", name=f"qkT{mc}"))
    vx_sb = []   # 16 tiles [128, 4*65] bf16: per head 64 v-cols + ones col
    aT_sb = [p_aT.tile([128, S], BF16, tag=f"aT{i}", name=f"aT{i}")
             for i in range(2)]

    # ---- phase A: projections + rope ----
    with tc.tile_pool(name="ps_qk", bufs=2, space="PSUM") as ps_qk, \
         tc.tile_pool(name="ps_v", bufs=2, space="PSUM") as ps_v:
        with nc.named_scope("qk_proj"):
            for mc in (0, 2, 1, 3):   # q01, k01 first so head 0 can start early
                for ns in range(4):
                    qk_ps = ps_qk.tile([128, 512], F32, tag="qk")
                    for kc in range(8):
                        nc.tensor.matmul(
                            qk_ps[:, :],
                            wqk_sb[kc][:, ts(mc, 128)],
                            x_sb[kc][:, ts(ns, 512)],
                            start=(kc == 0), stop=(kc == 7))
                    raw = p_tmp.tile([128, 512], F32, tag="raw")
                    nc.vector.tensor_scalar_add(raw[:, :], qk_ps[:, :], bqk_sb[mc][:, :])
                    # swap32: exchange adjacent 32-partition blocks (evens<->odds)
                    swp = p_tmp.tile([128, 512], F32, tag="swp")
                    for blk, src in ((0, 32), (32, 0), (64, 96), (96, 64)):
                        nc.sync.dma_start(swp[blk:blk + 32, :],
                                          raw[src:src + 32, :])
                    t1 = p_tmp.tile([128, 512], F32, tag="t1")
                    nc.vector.tensor_mul(t1[:, :], swp[:, :], tabQ[:, ts(ns, 512)])
                    t2 = p_tmp.tile([128, 512], F32, tag="t2")
                    nc.vector.tensor_mul(t2[:, :], raw[:, :], tabP[:, ts(ns, 512)])
                    nc.vector.tensor_add(qk_sb[mc][:, ts(ns, 512)], t1[:, :], t2[:, :])
        with nc.named_scope("v_proj"):
            for st in range(NT):
                v_ps = ps_v.tile([128, 256], F32, tag="v")
                for kc in range(8):
                    nc.tensor.matmul(
                        v_ps[:, :],
                        x_sb[kc][:, ts(st, 128)],
                        wv_sb[kc][:, :],
                        start=(kc == 0), stop=(kc == 7))
                vx_t = p_vx.tile([128, HPC * 65], BF16, tag=f"vx{st}")
                vv = vx_t.rearrange("p (h c) -> p h c", c=65)
                nc.vector.memset(vv[:, :, 64:65], 1.0)
                nc.vector.tensor_add(
                    vv[:, :, 0:64],
                    v_ps.rearrange("p (h c) -> p h c", c=64)[:, :, :],
                    bvb_sb.rearrange("p (h c) -> p h c", c=64)[:, :, :])
                vx_sb.append(vx_t)

    # ---- phase B: attention ----
    with tc.tile_pool(name="ps_s", bufs=2, space="PSUM") as ps_s, \
         tc.tile_pool(name="ps_o", bufs=1, space="PSUM") as ps_o:
        for h in range(HPC):
            hp, hr = h // 2, h % 2
            qT = qk_sb[hp]
            kT = qk_sb[2 + hp]
            with nc.named_scope(f"attn_h{h}"):
                oT = ps_o.tile([65, S], F32, tag="oT")
                for j in range(NT):
                    chunks = []
                    if j < 8:
                        chunks.append((j, 8 - j))
                    q0b = max(j, 8)
                    chunks.append((q0b, NT - q0b))
                    for (q0, qn) in chunks:
                        w = qn * 128
                        s_ps = ps_s.tile([128, 1024], F32, tag="s")
                        base = 64 * hr
                        for n0 in range(0, w, 512):
                            nn = min(512, w - n0)
                            nc.tensor.matmul(
                                s_ps[:, ds(n0, nn)],
                                kT[base:base + 64, ts(j, 128)],
                                qT[base:base + 64, ds(q0 * 128 + n0, nn)],
                                start=True, stop=True)
                        if q0 == j:
                            nc.vector.tensor_add(
                                s_ps[:, 0:128], s_ps[:, 0:128], maskneg[:, :])
                        p_t = p_p.tile([128, 1024], BF16, tag="p")
                        nc.scalar.activation(
                            p_t[:, 0:w], s_ps[:, 0:w], AF.Exp, scale=SCALE)
                        # PV accumulation, split at oT 512-col bank boundaries
                        c0 = q0 * 128
                        pos = c0
                        while pos < c0 + w:
                            nxt = min((pos // 512 + 1) * 512, c0 + w)
                            bank = pos // 512
                            nc.tensor.matmul(
                                oT[:, ds(pos, nxt - pos)],
                                vx_sb[j][:, ds(65 * h, 65)],
                                p_t[:, ds(pos - c0, nxt - pos)],
                                start=(j == 0), stop=(j == 4 * bank + 3),
                                skip_group_check=True)
                            pos = nxt
                # normalize: aT[head dims, q] = oT[0:64] * (1 / oT[64])
                r_t = p_r.tile([1, S], F32, tag="r")
                nc.vector.reciprocal(r_t[:, :], oT[64:65, :])
                rb_t = p_r.tile([64, S], F32, tag="rb")
                nc.gpsimd.partition_broadcast(rb_t[:, :], r_t[:, :])
                nc.vector.tensor_mul(
                    aT_sb[hp][64 * hr:64 * hr + 64, :], oT[0:64, :], rb_t[:, :])

    # ---- phase C: output projection ----
    with tc.tile_pool(name="ps_y", bufs=4, space="PSUM") as ps_y:
        with nc.named_scope("y_proj"):
            for qt in range(NT):
                for nh in range(2):
                    y_ps = ps_y.tile([128, 512], F32, tag="y")
                    for kc in range(2):
                        nc.tensor.matmul(
                            y_ps[:, :],
                            aT_sb[kc][:, ts(qt, 128)],
                            wout_sb[kc][:, ts(nh, 512)],
                            start=(kc == 0), stop=(kc == 1))
                    y_sb = p_y.tile([128, 512], F32, tag="ysb")
                    if nh == 0:
                        nc.vector.tensor_copy(y_sb[:, :], y_ps[:, :])
                    else:
                        nc.scalar.copy(y_sb[:, :], y_ps[:, :])
                    nc.sync.dma_start(y[ts(qt, 128), ts(nh, 512)], y_sb[:, :])


def build():
    nc = bacc.Bacc("TRN2", target_bir_lowering=False, debug=False,
                   num_devices=NCORES)
    xT = nc.dram_tensor("xT", [D, S], BF16, kind="ExternalInput").ap()
    wqk = nc.dram_tensor("wqk", [D, 512], BF16, kind="ExternalInput").ap()
    wv = nc.dram_tensor("wv", [D, 256], BF16, kind="ExternalInput").ap()
    wout = nc.dram_tensor("wout", [256, D], BF16, kind="ExternalInput").ap()
    bqk = nc.dram_tensor("bqk", [512], F32, kind="ExternalInput").ap()
    bvb = nc.dram_tensor("bvb", [128, 256], F32, kind="ExternalInput").ap()
    ropeP = nc.dram_tensor("ropeP", [128, S], F32, kind="ExternalInput").ap()
    ropeQ = nc.dram_tensor("ropeQ", [128, S], F32, kind="ExternalInput").ap()
    y = nc.dram_tensor("y", [S, D], F32, kind="ExternalOutput").ap()

    from contextlib import ExitStack
    with tile.TileContext(nc) as tc:
        with ExitStack() as ctx:
            _body(ctx, tc, (xT, wqk, wv, wout, bqk, bvb, ropeP, ropeQ), (y,))
    nc.compile()
    return nc


_EVEN_ODD = np.concatenate([np.arange(0, HD, 2), np.arange(1, HD, 2)])


def make_core_inputs(x, rope_cos, rope_sin, Wqkv, bqkv, Wout, bout, core):
    """Build the per-core device input map (numpy, host-side sharding)."""
    b, g = core // HPC, core % HPC
    heads = [HPC * g + i for i in range(HPC)]
    bf = ml_dtypes.bfloat16

    xT = np.ascontiguousarray(x[b].T).astype(bf)

    # wqk columns: [q01, q23, k01, k23]; within each head [evens, odds]
    qcols, kcols = [], []
    for h in heads:
        qcols.append(Wqkv[:, 0 * D + 64 * h + _EVEN_ODD])
        kcols.append(Wqkv[:, 1 * D + 64 * h + _EVEN_ODD])
    wqk_np = np.concatenate(
        [qcols[0], qcols[1], qcols[2], qcols[3],
         kcols[0], kcols[1], kcols[2], kcols[3]], axis=1)
    # reorder to [q01(128), q23(128), k01(128), k23(128)]
    wqk_np = np.concatenate(
        [wqk_np[:, 0:128], wqk_np[:, 128:256],
         wqk_np[:, 256:384], wqk_np[:, 384:512]], axis=1)
    bq = [bqkv[0 * D + 64 * h + _EVEN_ODD] for h in heads]
    bk = [bqkv[1 * D + 64 * h + _EVEN_ODD] for h in heads]
    bqk_np = np.concatenate([bq[0], bq[1], bq[2], bq[3],
                             bk[0], bk[1], bk[2], bk[3]])

    wv_np = np.concatenate(
        [Wqkv[:, 2 * D + 64 * h:2 * D + 64 * h + 64] for h in heads], axis=1)
    bv = np.concatenate(
        [bqkv[2 * D + 64 * h:2 * D + 64 * h + 64] for h in heads])
    bvb_np = np.tile(bv[None, :], (128, 1)).astype(np.float32)

    wout_np = np.concatenate(
        [Wout[64 * h:64 * h + 64, :] for h in heads], axis=0)

    cosT = np.ascontiguousarray(rope_cos.T).astype(np.float32)  # [32, S]
    sinT = np.ascontiguousarray(rope_sin.T).astype(np.float32)
    ropeP_np = np.tile(np.concatenate([cosT, cosT], axis=0), (2, 1))
    ropeQ_np = np.tile(np.concatenate([-sinT, sinT], axis=0), (2, 1))

    return {
        "xT": xT,
        "wqk": np.ascontiguousarray(wqk_np).astype(bf),
        "wv": np.ascontiguousarray(wv_np).astype(bf),
        "wout": np.ascontiguousarray(wout_np).astype(bf),
        "bqk": bqk_np.astype(np.float32),
        "bvb": bvb_np,
        "ropeP": np.ascontiguousarray(ropeP_np),
        "ropeQ": np.ascontiguousarray(ropeQ_np),
    }


_NC_CACHE = None


def kernel(x, rope_cos, rope_sin, Wqkv, bqkv, Wout, bout):
    global _NC_CACHE, LAST_RESULTS
    x = np.asarray(x, dtype=np.float32)
    rope_cos = np.asarray(rope_cos, dtype=np.float32)
    rope_sin = np.asarray(rope_sin, dtype=np.float32)
    Wqkv = np.asarray(Wqkv, dtype=np.float32)
    bqkv = np.asarray(bqkv, dtype=np.float32)
    Wout = np.asarray(Wout, dtype=np.float32)
    bout = np.asarray(bout, dtype=np.float32)

    if _NC_CACHE is None:
        _NC_CACHE = build()
    nc = _NC_CACHE

    in_maps = [
        make_core_inputs(x, rope_cos, rope_sin, Wqkv, bqkv, Wout, bout, c)
        for c in range(NCORES)
    ]
    res = run_bass_kernel_spmd(nc, in_maps, core_ids=list(range(NCORES)),
                               trace=TRACE)
    LAST_RESULTS = res

    out = np.zeros((B, S, D), dtype=np.float32)
    for c in range(NCORES):
        out[c // HPC] += res.results[c]["y"]
    out += bout[None, None, :]
    return out


# revision 7
# speedup vs baseline: 1.0664x; 1.0664x over previous
# Causal self-attention kernel for 8 Trainium2 NeuronCores (Bass/Tile).
#
# Sharding: core c -> batch b = c//4, head group g = c%4 (heads 4g..4g+3).
# Each core computes qkv projection for its batch restricted to its heads
# (column-sharded Wqkv), rope, causal flash attention for its 4 heads, and a
# row-sharded output projection producing a partial [S, D] f32 output.  Host
# sums the 4 partials per batch and adds bout.
#
# Device-side layout notes:
#  * All matmul inputs are bf16 (fp32 matmul is 4x slower on the PE); all
#    accumulation is f32 in PSUM.
#  * x is pre-transposed on host to xT [D, S] so the contraction dim (D) lands
#    on SBUF partitions without any on-device transpose.
#  * q/k are produced directly transposed: qT/kT [head dims, S].  Within each
#    head the 64 dims are permuted to [evens(32), odds(32)] so rope becomes
#    rot = x*P + swap32(x)*Q with per-row tables P/Q (host-built) and swap32
#    done by 2 SBUF->SBUF DMAs.
#  * Scores are computed transposed, sT[k, q] = k . q, via two accumulating
#    K=32 matmuls (even dims + odd dims) in distinct PE row groups.
#  * Softmax without max-subtraction (scores ~ N(0,1), exp is safe in f32):
#    p = exp(s/8) straight out of PSUM on the scalar engine (bf16 out).
#  * v_ext [k, 65] carries a ones-column so the PV matmul accumulates the
#    softmax denominator as row 64 of oT [65, q].  Normalization:
#    reciprocal (DVE) -> partition_broadcast (GpSimd) -> multiply (DVE).
#  * Output projection: y[q, n] = sum_d aT[d, q] * Wout[d, n], q-tiled.

import numpy as np
import ml_dtypes

import concourse.bass as bass
import concourse.tile as tile
import concourse.mybir as mybir
from concourse import bacc
from concourse.bass import ts, ds
from concourse.bass_utils import run_bass_kernel_spmd

BF16 = mybir.dt.bfloat16
F32 = mybir.dt.float32
AF = mybir.ActivationFunctionType
ALU = mybir.AluOpType

B, S, D = 2, 2048, 1024
H, HD = 16, 64
NCORES = 8
HPC = 4            # heads per core
NT = S // 128      # 16 seq tiles
SCALE = HD ** -0.5
MASK_NEG = -30000.0

# Module-level knobs / results (used by test.py).
TRACE = False
LAST_RESULTS = None


def _body(ctx, tc, ins, outs):
    nc = tc.nc
    xT, wqk, wv, wout, bqk, bvb, ropeP, ropeQ = ins
    (y,) = outs

    # ---- persistent SBUF pools ----
    p_x = ctx.enter_context(tc.tile_pool(name="x", bufs=1))
    p_w = ctx.enter_context(tc.tile_pool(name="w", bufs=1))
    p_cst = ctx.enter_context(tc.tile_pool(name="cst", bufs=1))
    p_qk = ctx.enter_context(tc.tile_pool(name="qk", bufs=1))
    p_vx = ctx.enter_context(tc.tile_pool(name="vx", bufs=1))
    p_aT = ctx.enter_context(tc.tile_pool(name="aT", bufs=1))
    p_tmp = ctx.enter_context(tc.tile_pool(name="tmp", bufs=2))
    p_p = ctx.enter_context(tc.tile_pool(name="p", bufs=4))
    p_r = ctx.enter_context(tc.tile_pool(name="r", bufs=2))
    p_y = ctx.enter_context(tc.tile_pool(name="y", bufs=4))

    # ---- load inputs ----
    x_sb = []
    for kc in range(8):
        t = p_x.tile([128, S], BF16, tag=f"x{kc}")
        nc.sync.dma_start(t[:, :], xT[ts(kc, 128), :])
        x_sb.append(t)
    wqk_sb = []
    for kc in range(8):
        t = p_w.tile([128, 512], BF16, tag=f"wqk{kc}")
        nc.sync.dma_start(t[:, :], wqk[ts(kc, 128), :])
        wqk_sb.append(t)
    wv_sb = []
    for kc in range(8):
        t = p_w.tile([128, 256], BF16, tag=f"wv{kc}")
        nc.sync.dma_start(t[:, :], wv[ts(kc, 128), :])
        wv_sb.append(t)
    wout_sb = []
    for kc in range(2):
        t = p_w.tile([128, D], BF16, tag=f"wout{kc}")
        nc.sync.dma_start(t[:, :], wout[ts(kc, 128), :])
        wout_sb.append(t)
    tabP = p_cst.tile([128, S], F32, tag="tabP")
    nc.sync.dma_start(tabP[:, :], ropeP[:, :])
    tabQ = p_cst.tile([128, S], F32, tag="tabQ")
    nc.sync.dma_start(tabQ[:, :], ropeQ[:, :])
    bqk_sb = []
    for mc in range(4):
        t = p_cst.tile([128, 1], F32, tag=f"bqk{mc}")
        nc.sync.dma_start(t[:, :], bqk[ts(mc, 128)].rearrange("(p o) -> p o", o=1))
        bqk_sb.append(t)
    bvb_sb = p_cst.tile([128, 256], F32, tag="bvb")
    nc.sync.dma_start(bvb_sb[:, :], bvb[:, :])

    qk_sb = []   # [q01, q23, k01, k23], bf16 [128, S] each (post-rope)
    for mc in range(4):
        qk_sb.append(p_qk.tile([128, S], BF16, tag=f"qkT{mc}", name=f"qkT{mc}"))
    vx_sb = []   # 16 tiles [128, 4*65] bf16: per head 64 v-cols + ones col
    aT_sb = [p_aT.tile([128, S], BF16, tag=f"aT{i}", name=f"aT{i}")
             for i in range(2)]

    # ---- phase A: projections + rope ----
    with tc.tile_pool(name="ps_qk", bufs=2, space="PSUM") as ps_qk, \
         tc.tile_pool(name="ps_v", bufs=2, space="PSUM") as ps_v:
        with nc.named_scope("qk_proj"):
            for mc in (0, 2, 1, 3):   # q01, k01 first so head 0 can start early
                for ns in range(4):
                    qk_ps = ps_qk.tile([128, 512], F32, tag="qk")
                    for kc in range(8):
                        nc.tensor.matmul(
                            qk_ps[:, :],
                            wqk_sb[kc][:, ts(mc, 128)],
                            x_sb[kc][:, ts(ns, 512)],
                            start=(kc == 0), stop=(kc == 7))
                    raw = p_tmp.tile([128, 512], F32, tag="raw")
                    nc.vector.tensor_scalar_add(raw[:, :], qk_ps[:, :], bqk_sb[mc][:, :])
                    # swap32: exchange adjacent 32-partition blocks (evens<->odds)
                    swp = p_tmp.tile([128, 512], F32, tag="swp")
                    for blk, src in ((0, 32), (32, 0), (64, 96), (96, 64)):
                        nc.sync.dma_start(swp[blk:blk + 32, :],
                                          raw[src:src + 32, :])
                    t1 = p_tmp.tile([128, 512], F32, tag="t1")
                    nc.vector.tensor_mul(t1[:, :], swp[:, :], tabQ[:, ts(ns, 512)])
                    t2 = p_tmp.tile([128, 512], F32, tag="t2")
                    nc.vector.tensor_mul(t2[:, :], raw[:, :], tabP[:, ts(ns, 512)])
                    nc.vector.tensor_add(qk_sb[mc][:, ts(ns, 512)], t1[:, :], t2[:, :])
        with nc.named_scope("v_proj"):
            for st in range(NT):
                v_ps = ps_v.tile([128, 256], F32, tag="v")
                for kc in range(8):
                    nc.tensor.matmul(
                        v_ps[:, :],
                        x_sb[kc][:, ts(st, 128)],
                        wv_sb[kc][:, :],
                        start=(kc == 0), stop=(kc == 7))
                vx_t = p_vx.tile([128, HPC * 65], BF16, tag=f"vx{st}")
                vv = vx_t.rearrange("p (h c) -> p h c", c=65)
                nc.vector.memset(vv[:, :, 64:65], 1.0)
                nc.vector.tensor_add(
                    vv[:, :, 0:64],
                    v_ps.rearrange("p (h c) -> p h c", c=64)[:, :, :],
                    bvb_sb.rearrange("p (h c) -> p h c", c=64)[:, :, :])
                vx_sb.append(vx_t)

    # ---- phase B: attention ----
    with tc.tile_pool(name="ps_s", bufs=2, space="PSUM") as ps_s, \
         tc.tile_pool(name="ps_o", bufs=1, space="PSUM") as ps_o:
        for h in range(HPC):
            hp, hr = h // 2, h % 2
            qT = qk_sb[hp]
            kT = qk_sb[2 + hp]
            with nc.named_scope(f"attn_h{h}"):
                oT = ps_o.tile([65, S], F32, tag="oT")
                for j in range(NT):
                    chunks = []
                    if j < 8:
                        chunks.append((j, 8 - j))
                    q0b = max(j, 8)
                    chunks.append((q0b, NT - q0b))
                    for (q0, qn) in chunks:
                        w = qn * 128
                        s_ps = ps_s.tile([128, 1024], F32, tag="s")
                        base = 64 * hr
                        for n0 in range(0, w, 512):
                            nn = min(512, w - n0)
                            nc.tensor.matmul(
                                s_ps[:, ds(n0, nn)],
                                kT[base:base + 64, ts(j, 128)],
                                qT[base:base + 64, ds(q0 * 128 + n0, nn)],
                                start=True, stop=True)
                        p_t = p_p.tile([128, 1024], BF16, tag="p")
                        nc.scalar.activation(
                            p_t[:, 0:w], s_ps[:, 0:w], AF.Exp, scale=SCALE)
                        if q0 == j:
                            # zero the upper-triangular (k>q) part of the
                            # diagonal tile, in place, off the PE/ACT path
                            nc.gpsimd.affine_select(
                                p_t[:, 0:128], p_t[:, 0:128],
                                pattern=[[1, 128]], compare_op=ALU.is_ge,
                                fill=0.0, base=0, channel_multiplier=-1)
                        # PV accumulation, split at oT 512-col bank boundaries
                        c0 = q0 * 128
                        pos = c0
                        while pos < c0 + w:
                            nxt = min((pos // 512 + 1) * 512, c0 + w)
                            bank = pos // 512
                            nc.tensor.matmul(
                                oT[:, ds(pos, nxt - pos)],
                                vx_sb[j][:, ds(65 * h, 65)],
                                p_t[:, ds(pos - c0, nxt - pos)],
                                start=(j == 0), stop=(j == 4 * bank + 3),
                                skip_group_check=True)
                            pos = nxt
                # normalize: aT[head dims, q] = oT[0:64] * (1 / oT[64])
                lg_t = p_r.tile([1, S], F32, tag="lg")
                nc.scalar.activation(lg_t[:, :], oT[64:65, :], AF.Ln)
                r_t = p_r.tile([1, S], F32, tag="r")
                nc.scalar.activation(r_t[:, :], lg_t[:, :], AF.Exp, scale=-1.0)
                rb_t = p_r.tile([64, S], F32, tag="rb")
                nc.gpsimd.partition_broadcast(rb_t[:, :], r_t[:, :])
                nc.vector.tensor_mul(
                    aT_sb[hp][64 * hr:64 * hr + 64, :], oT[0:64, :], rb_t[:, :])

    # ---- phase C: output projection ----
    with tc.tile_pool(name="ps_y", bufs=4, space="PSUM") as ps_y:
        with nc.named_scope("y_proj"):
            for qt in range(NT):
                for nh in range(2):
                    y_ps = ps_y.tile([128, 512], F32, tag="y")
                    for kc in range(2):
                        nc.tensor.matmul(
                            y_ps[:, :],
                            aT_sb[kc][:, ts(qt, 128)],
                            wout_sb[kc][:, ts(nh, 512)],
                            start=(kc == 0), stop=(kc == 1))
                    y_sb = p_y.tile([128, 512], F32, tag="ysb")
                    nc.vector.tensor_copy(y_sb[:, :], y_ps[:, :])
                    nc.sync.dma_start(y[ts(qt, 128), ts(nh, 512)], y_sb[:, :])


def build():
    nc = bacc.Bacc("TRN2", target_bir_lowering=False, debug=False,
                   num_devices=NCORES)
    xT = nc.dram_tensor("xT", [D, S], BF16, kind="ExternalInput").ap()
    wqk = nc.dram_tensor("wqk", [D, 512], BF16, kind="ExternalInput").ap()
    wv = nc.dram_tensor("wv", [D, 256], BF16, kind="ExternalInput").ap()
    wout = nc.dram_tensor("wout", [256, D], BF16, kind="ExternalInput").ap()
    bqk = nc.dram_tensor("bqk", [512], F32, kind="ExternalInput").ap()
    bvb = nc.dram_tensor("bvb", [128, 256], F32, kind="ExternalInput").ap()
    ropeP = nc.dram_tensor("ropeP", [128, S], F32, kind="ExternalInput").ap()
    ropeQ = nc.dram_tensor("ropeQ", [128, S], F32, kind="ExternalInput").ap()
    y = nc.dram_tensor("y", [S, D], F32, kind="ExternalOutput").ap()

    from contextlib import ExitStack
    with tile.TileContext(nc) as tc:
        with ExitStack() as ctx:
            _body(ctx, tc, (xT, wqk, wv, wout, bqk, bvb, ropeP, ropeQ), (y,))
    nc.compile()
    return nc


_EVEN_ODD = np.concatenate([np.arange(0, HD, 2), np.arange(1, HD, 2)])


def make_core_inputs(x, rope_cos, rope_sin, Wqkv, bqkv, Wout, bout, core):
    """Build the per-core device input map (numpy, host-side sharding)."""
    b, g = core // HPC, core % HPC
    heads = [HPC * g + i for i in range(HPC)]
    bf = ml_dtypes.bfloat16

    xT = np.ascontiguousarray(x[b].T).astype(bf)

    # wqk columns: [q01, q23, k01, k23]; within each head [evens, odds]
    qcols, kcols = [], []
    for h in heads:
        qcols.append(Wqkv[:, 0 * D + 64 * h + _EVEN_ODD])
        kcols.append(Wqkv[:, 1 * D + 64 * h + _EVEN_ODD])
    wqk_np = np.concatenate(
        [qcols[0], qcols[1], qcols[2], qcols[3],
         kcols[0], kcols[1], kcols[2], kcols[3]], axis=1)
    # reorder to [q01(128), q23(128), k01(128), k23(128)]
    wqk_np = np.concatenate(
        [wqk_np[:, 0:128], wqk_np[:, 128:256],
         wqk_np[:, 256:384], wqk_np[:, 384:512]], axis=1)
    bq = [bqkv[0 * D + 64 * h + _EVEN_ODD] for h in heads]
    bk = [bqkv[1 * D + 64 * h + _EVEN_ODD] for h in heads]
    bqk_np = np.concatenate([bq[0], bq[1], bq[2], bq[3],
                             bk[0], bk[1], bk[2], bk[3]])

    wv_np = np.concatenate(
        [Wqkv[:, 2 * D + 64 * h:2 * D + 64 * h + 64] for h in heads], axis=1)
    bv = np.concatenate(
        [bqkv[2 * D + 64 * h:2 * D + 64 * h + 64] for h in heads])
    bvb_np = np.tile(bv[None, :], (128, 1)).astype(np.float32)

    wout_np = np.concatenate(
        [Wout[64 * h:64 * h + 64, :] for h in heads], axis=0)

    cosT = np.ascontiguousarray(rope_cos.T).astype(np.float32)  # [32, S]
    sinT = np.ascontiguousarray(rope_sin.T).astype(np.float32)
    ropeP_np = np.tile(np.concatenate([cosT, cosT], axis=0), (2, 1))
    ropeQ_np = np.tile(np.concatenate([-sinT, sinT], axis=0), (2, 1))

    return {
        "xT": xT,
        "wqk": np.ascontiguousarray(wqk_np).astype(bf),
        "wv": np.ascontiguousarray(wv_np).astype(bf),
        "wout": np.ascontiguousarray(wout_np).astype(bf),
        "bqk": bqk_np.astype(np.float32),
        "bvb": bvb_np,
        "ropeP": np.ascontiguousarray(ropeP_np),
        "ropeQ": np.ascontiguousarray(ropeQ_np),
    }


_NC_CACHE = None


def kernel(x, rope_cos, rope_sin, Wqkv, bqkv, Wout, bout):
    global _NC_CACHE, LAST_RESULTS
    x = np.asarray(x, dtype=np.float32)
    rope_cos = np.asarray(rope_cos, dtype=np.float32)
    rope_sin = np.asarray(rope_sin, dtype=np.float32)
    Wqkv = np.asarray(Wqkv, dtype=np.float32)
    bqkv = np.asarray(bqkv, dtype=np.float32)
    Wout = np.asarray(Wout, dtype=np.float32)
    bout = np.asarray(bout, dtype=np.float32)

    if _NC_CACHE is None:
        _NC_CACHE = build()
    nc = _NC_CACHE

    in_maps = [
        make_core_inputs(x, rope_cos, rope_sin, Wqkv, bqkv, Wout, bout, c)
        for c in range(NCORES)
    ]
    res = run_bass_kernel_spmd(nc, in_maps, core_ids=list(range(NCORES)),
                               trace=TRACE)
    LAST_RESULTS = res

    out = np.zeros((B, S, D), dtype=np.float32)
    for c in range(NCORES):
        out[c // HPC] += res.results[c]["y"]
    out += bout[None, None, :]
    return out
